# revision 47
# baseline (speedup 1.0000x reference)
"""Trainium2 Bass kernel for ExtractRelevantPatches (pool -> top-k -> gather).

Full-input contract: kernel(heatmap [64,448,448,1] f32, image [64,448,448,3] f32)
-> [1344, 64, 64, 3] f32.

Sharding: pure data-parallel over batch; 8 batches per NeuronCore, 8 cores.

Per-core algorithm (raw Bass, explicit semaphores), v8 — pipelined over 4
groups of 2 batches with a software-pipelined engine schedule:

  Per group g (batches 2g, 2g+1; b' = par):
  1. Heatmap -> SBUF [128, 7, 448], partition p = 64*par + r. The two
     partition halves ride DIFFERENT HWDGE rings (par0 on SP/sync, par1 on
     ACT/scalar) so even- and odd-port halves of the SBUF fabric stream
     concurrently (one 64-partition DMA reaches only half the AXI ports).
  2. One DVE reduce_sum over 64-col groups -> red [128, 49].
  3. PE: selector matmul F2 -> psV [2, 49] pooled sums (br-major), and the
     same matmul over a transposed rhs view -> psVT (bc-major).
  4. ACT copies psV/psVT to SBUF (ACT has its own SBUF ports, so nothing
     here contends with gather desc-gen, which holds the DVE<->GpSimd
     shared-port lock for ~7.6us per chunk).
  5. DVE top-24 (3 rounds), keep 21: per round max + find_index on V
     (pos1 = 7br+bc) + find_index on VT (pos2 = 7bc+br) + match_replace.
  6. ACT converts pos1/pos2 u32 -> f32; PE broadcasts
     base = 65.3125*pos1 - 9.1875*pos2 = 448*br + bc (exact in f32) to all
     128 partitions via accumulating matmuls with pre-scaled one-hot
     selectors -> psD [128, 42].
  7. DVE adds psD (x4 stride-0 bcast) to the f32 static table -> int16
     idx16 slice. The DVE stream is software-pipelined: reduce(g) runs
     in the ACT/PE round-trip window of group g-1, i.e.
     [reduce g][idx16-add g-1][topk g].
  8. dma_gather chunks on 4 SWDGE queues (g0 graduated 128..896 for fast
     ramp, then uniform 896); one store per chunk, alternating between
     the two HWDGE rings.
"""

import numpy as np

_N_CORES = 8
_B = 64
_B_LOC = _B // _N_CORES  # 8
_PATCH = 64
_GRID = 7
_NPATCH = 21
_PROW = _PATCH * 3            # 192 elements per patch-row
_OUT_ROWS_LOC = _B_LOC * _NPATCH  # 168
_NGRP = 4                     # batch groups per core
_BG = _B_LOC // _NGRP         # 2 batches per group

# gather chunk sizes per group (multiples of 128; each sums to 2688).
# Early groups use small chunks (low desc-gen latency ~3.6us) since the
# queues are latency-bound then; later groups use 896-idx chunks for
# better desc-gen throughput per Q7 core (fixed cost amortization) once
# the queues run with backlog.
_CHUNKS_BY_G = [
    [128, 256, 384, 384, 384, 384, 384, 384],
    [384] * 7,
    [384] * 7,
    [384] * 7,
]

_nc_cache = None


def build_program():
    """Build the per-core SPMD Bass program (cached)."""
    global _nc_cache
    if _nc_cache is not None:
        return _nc_cache

    import concourse.bass as bass
    import concourse.bacc as bacc
    import concourse.mybir as mybir

    f32 = mybir.dt.float32
    i16 = mybir.dt.int16
    u32 = mybir.dt.uint32
    X = mybir.AxisListType.X
    Op = mybir.AluOpType

    nc = bacc.Bacc(num_swdge_queues=4)

    hm_in = nc.declare_dram_parameter(
        "heatmap", [_B_LOC, 448, 448, 1], f32, isOutput=False)
    img_in = nc.declare_dram_parameter(
        "image", [_B_LOC, 448, 448, 3], f32, isOutput=False)
    out_t = nc.declare_dram_parameter(
        "out", [_OUT_ROWS_LOC, _PATCH, _PATCH, 3], f32, isOutput=True)

    # --- one packed inline constant [128, 1186] f32 -----------------------
    # cols 0:2      F2 selector  (F2[p, i] = 1 iff i == p//64)
    # cols 2:258    A_bl = 65.3125 * E2_bl  (E2_bl[p, i] = 1 iff p == bl)
    # cols 258:514  B_bl = -9.1875 * E2_bl
    #   base = 448*br + bc = 65.3125*pos1 - 9.1875*pos2 exactly in f32
    # cols 514:1186 static gather-index table: position i = R at
    #   [R%16, R//16]; col s: term = 7*(R%16) + 112*(s%4) + 3136*(s//84)
    pk = np.zeros((128, 1186), dtype=np.float32)
    pk[:64, 0] = 1.0
    pk[64:, 1] = 1.0
    pk[0, 2:130] = 65.3125
    pk[1, 130:258] = 65.3125
    pk[0, 258:386] = -9.1875
    pk[1, 386:514] = -9.1875
    s_ar = np.arange(672, dtype=np.int64)
    w_ar = np.arange(16, dtype=np.int64)
    st = (112 * (s_ar[None, :] % 4) + 7 * w_ar[:, None]
          + 3136 * (s_ar[None, :] // 84)).astype(np.float32)
    pk[:, 514:1186] = np.tile(st, (8, 1))
    pk_const = nc.inline_tensor(pk, name="pk_const")
    _ST0 = 514  # sttab column offset within pk

    # --- DRAM views -------------------------------------------------------
    img_rows = (img_in[:]
                .rearrange("b r c ch -> (b r c ch)")
                .rearrange("(n e) -> n e", e=_PROW))

    out_pc = (out_t[:]
              .rearrange("r a b c -> (r a b c)")
              .rearrange("(n e) -> n e", e=_PROW)
              .rearrange("(c p) e -> p c e", p=128))

    # heatmap per-group views: [par, 64, 7, 448]
    hm_src = []
    for g in range(_NGRP):
        hm_src.append(
            hm_in[2 * g:2 * (g + 1)]
            .rearrange("par (br r) c one -> par r br (c one)", r=64))

    # per-chunk geometry: (group, idx16 col offset, idx16 col width,
    #                      GT col offset, GT col width, num idxs, queue,
    #                      nth-on-queue)
    chunk_geo = []
    qcount = [0, 0, 0, 0]
    cidx = 0
    for g in range(_NGRP):
        off16 = 168 * g
        offGT = 21 * g
        for n in _CHUNKS_BY_G[g]:
            q = cidx % 4
            chunk_geo.append((g, off16, n // 16, offGT, n // 128, n, q,
                              qcount[q]))
            qcount[q] += 1
            off16 += n // 16
            offGT += n // 128
            cidx += 1
    _NCHUNK = len(chunk_geo)  # 14

    from contextlib import ExitStack

    with ExitStack() as ctx:
        e = ctx.enter_context
        hm = [e(nc.sbuf_tensor(f"hm{g}", [128, 7, 448], f32))
              for g in range(_NGRP)]
        red = [e(nc.sbuf_tensor(f"red{g}", [128, 49], f32))
               for g in range(_NGRP)]
        pk_sb = e(nc.sbuf_tensor("pk_sb", [128, 1186], f32))
        vwork = [e(nc.sbuf_tensor(f"vwork{g}", [2, 49], f32))
                 for g in range(_NGRP)]
        vwork2 = [e(nc.sbuf_tensor(f"vwork2_{g}", [2, 49], f32))
                  for g in range(_NGRP)]
        m8 = [e(nc.sbuf_tensor(f"m8_{g}", [2, 8], f32)) for g in range(_NGRP)]
        idx_u = [e(nc.sbuf_tensor(f"idx_u{g}", [2, 24], u32))
                 for g in range(_NGRP)]
        idx_u2 = [e(nc.sbuf_tensor(f"idx_u2_{g}", [2, 24], u32))
                  for g in range(_NGRP)]
        pos1f = [e(nc.sbuf_tensor(f"pos1f{g}", [2, _NPATCH], f32))
                 for g in range(_NGRP)]
        pos2f = [e(nc.sbuf_tensor(f"pos2f{g}", [2, _NPATCH], f32))
                 for g in range(_NGRP)]
        idx16 = e(nc.sbuf_tensor("idx16", [128, 672], i16))
        GT = e(nc.sbuf_tensor("GT", [128, 84, _PROW], f32))
        # single shared PSUM tensors: group g+1's producer matmuls are
        # ordered after group g's consumers via program order
        psV = e(nc.psum_tensor("psV", [2, 49], f32))
        psVT = e(nc.psum_tensor("psVT", [2, 49], f32))
        psD = e(nc.psum_tensor("psD", [128, 42], f32))

        s_ld = [e(nc.semaphore(f"s_ld{g}")) for g in range(_NGRP)]
        s_ld0b = e(nc.semaphore("s_ld0b"))  # group 0's second band-chunk
        s_red = e(nc.semaphore("s_red"))    # +1/group (DVE)
        s_mmV = e(nc.semaphore("s_mmV"))    # +2/group (PE)
        s_tk = e(nc.semaphore("s_tk"))      # +1/group (DVE top-k)
        s_base = e(nc.semaphore("s_base"))  # +2/group (ACT converts)
        s_mmD = e(nc.semaphore("s_mmD"))    # +2/group (PE broadcasts)
        s_idx = e(nc.semaphore("s_idx"))    # +1/group (DVE idx16 add)
        s_cst = e(nc.semaphore("s_cst"))
        s_gq = [e(nc.semaphore(f"s_gq{i}")) for i in range(_NCHUNK)]
        s_st = e(nc.semaphore("s_st"))
        block = e(nc.Block())

        @block.sync
        def _(sync):
            # par0 heatmap halves on the SP ring, then the even stores;
            # group 0 is split by band-range so its reduce can start early
            for g in range(_NGRP):
                for j, (b0, b1) in enumerate(
                        [(0, 4), (4, 7)] if g == 0 else [(0, 7)]):
                    sync.dma_start(
                        out=hm[g][0:64, b0:b1, :],
                        in_=hm_src[g][0:1, :, b0:b1, :]
                        .rearrange("one r br c -> (one r) br c"),
                    ).then_inc(s_ld0b if j else s_ld[g], 16)
            for c, (g, o16, w16, oGT, wGT, n, q, kq) in enumerate(chunk_geo):
                if c % 2:
                    continue  # odd chunks stored from the ACT ring
                sync.wait_ge(s_gq[c], 16)
                sync.dma_start(
                    out=out_pc[:, oGT:oGT + wGT, :],
                    in_=GT[:, oGT:oGT + wGT, :],
                ).then_inc(s_st, 16)
            sync.wait_ge(s_st, 16 * _NCHUNK)

        @block.scalar
        def _(sc):
            Act = mybir.ActivationFunctionType
            # par1 heatmap halves on the ACT ring (nothing else ahead)
            for g in range(_NGRP):
                for j, (b0, b1) in enumerate(
                        [(0, 4), (4, 7)] if g == 0 else [(0, 7)]):
                    sc.dma_start(
                        out=hm[g][64:128, b0:b1, :],
                        in_=hm_src[g][1:2, :, b0:b1, :]
                        .rearrange("one r br c -> (one r) br c"),
                    ).then_inc(s_ld0b if j else s_ld[g], 16)
            # per group: PSUM->SBUF pooled-sum copies, then after top-k the
            # u32->f32 index conversions; all on ACT's private SBUF ports
            for g in range(_NGRP):
                sc.wait_ge(s_tk, g + 1)
                sc.activation(
                    out=pos1f[g][:], in_=idx_u[g][:, :_NPATCH],
                    func=Act.Copy)
                sc.activation(
                    out=pos2f[g][:], in_=idx_u2[g][:, :_NPATCH],
                    func=Act.Copy)
                sc.drain().then_inc(s_base, 2)
            # the odd stores
            for c, (g, o16, w16, oGT, wGT, n, q, kq) in enumerate(chunk_geo):
                if c % 2 == 0:
                    continue
                sc.wait_ge(s_gq[c], 16)
                sc.dma_start(
                    out=out_pc[:, oGT:oGT + wGT, :],
                    in_=GT[:, oGT:oGT + wGT, :],
                ).then_inc(s_st, 16)

        def idx16_add(vector, g):
            vector.wait_ge(s_mmD, 2 * (g + 1))
            vector.tensor_tensor(
                out=idx16[:, 168 * g:168 * (g + 1)].rearrange(
                    "p (m q) -> p m q", q=4),
                in0=psD[:].rearrange(
                    "p (m one) -> p m one", one=1).to_broadcast(
                    [128, 42, 4]),
                in1=pk_sb[:, _ST0 + 168 * g:_ST0 + 168 * (g + 1)].rearrange(
                    "p (m q) -> p m q", q=4),
                op=Op.add)
            vector.drain().then_inc(s_idx, 1)

        @block.vector
        def _(vector):
            for g in range(_NGRP):
                if g == 0:
                    # group 0 arrives as two band-range pieces; reduce
                    # each as it lands
                    vector.wait_ge(s_ld[0], 32)
                    vector.reduce_sum(
                        out=red[0][:, 0:28],
                        in_=hm[0][:, 0:4, :].rearrange(
                            "p br (bc u) -> p (br bc) u", u=64),
                        axis=X,
                    )
                    vector.wait_ge(s_ld0b, 32)
                    vector.reduce_sum(
                        out=red[0][:, 28:49],
                        in_=hm[0][:, 4:7, :].rearrange(
                            "p br (bc u) -> p (br bc) u", u=64),
                        axis=X,
                    )
                else:
                    vector.wait_ge(s_ld[g], 32)
                    vector.reduce_sum(
                        out=red[g][:],
                        in_=hm[g][:].rearrange(
                            "p br (bc u) -> p (br bc) u", u=64),
                        axis=X,
                    )
                vector.drain().then_inc(s_red, 1)
                # group 3: group 2's idx16 add fits in the pooled-matmul
                # latency window right after this reduce
                if g == 3:
                    idx16_add(vector, g - 1)
                # top-24, keep 21, reading the pooled sums directly from
                # PSUM (psV br-major, psVT bc-major); find_index searches
                # for m8's values in the pristine arrays
                vector.wait_ge(s_mmV, 2 * (g + 1))
                cur = psV
                for r3 in range(3):
                    vector.max(out=m8[g][:], in_=cur[:])
                    vector.drain()
                    vector.max_index(
                        out=idx_u[g][:, 8 * r3:8 * r3 + 8], in_max=m8[g][:],
                        in_values=psV[:])
                    vector.max_index(
                        out=idx_u2[g][:, 8 * r3:8 * r3 + 8], in_max=m8[g][:],
                        in_values=psVT[:])
                    if r3 < 2:
                        nxt = vwork[g] if r3 == 0 else vwork2[g]
                        vector.match_replace(
                            out=nxt[:], in_to_replace=m8[g][:],
                            in_values=cur[:], imm_value=-1e30)
                        vector.drain()
                        cur = nxt
                vector.drain().then_inc(s_tk, 1)
                if g <= 1:
                    # inline for groups 0/1: release their gathers ASAP
                    # (queues are still latency-bound then)
                    idx16_add(vector, g)
            idx16_add(vector, _NGRP - 1)

        @block.tensor
        def _(tensor):
            for g in range(_NGRP):
                tensor.wait_ge(s_red, g + 1)
                if g == 0:
                    tensor.wait_ge(s_cst, 16)
                tensor.matmul(
                    out=psV[:],
                    lhsT=pk_sb[:, 0:2],
                    rhs=red[g][:],
                    start=True, stop=True,
                ).then_inc(s_mmV, 1)
                tensor.matmul(
                    out=psVT[:],
                    lhsT=pk_sb[:, 0:2],
                    rhs=red[g][:].rearrange("p (br bc) -> p bc br", bc=7),
                    start=True, stop=True,
                ).then_inc(s_mmV, 1)
                # broadcast base = 65.3125*pos1 - 9.1875*pos2 across
                # partitions via accumulating matmuls with pre-scaled
                # one-hot selectors
                tensor.wait_ge(s_base, 2 * (g + 1))
                for bl in range(2):
                    tensor.matmul(
                        out=psD[:, 21 * bl:21 * (bl + 1)],
                        lhsT=pk_sb[0:2, 2 + 128 * bl:2 + 128 * (bl + 1)],
                        rhs=pos1f[g][:],
                        start=True, stop=False)
                    tensor.matmul(
                        out=psD[:, 21 * bl:21 * (bl + 1)],
                        lhsT=pk_sb[0:2, 258 + 128 * bl:258 + 128 * (bl + 1)],
                        rhs=pos2f[g][:],
                        start=False, stop=True,
                    ).then_inc(s_mmD, 1)

        @block.gpsimd
        def _(g_):
            from concourse import library_config
            # the packed constant rides SWDGE so neither HWDGE ring pays a
            # small-DMA latency at its head; it must precede load_library,
            # whose ucode overlay blocks the gpsimd queue for ~14us
            g_.dma_start(out=pk_sb[:], in_=pk_const[:]).then_inc(s_cst, 16)
            g_.load_library(library_config.mlp)
            prev_g = -1
            for c, (g, o16, w16, oGT, wGT, n, q, kq) in enumerate(chunk_geo):
                if g != prev_g:
                    g_.wait_ge(s_idx, g + 1)
                    prev_g = g
                g_.dma_gather(
                    out_ap=GT[:, oGT:oGT + wGT, :],
                    in_ap=img_rows,
                    idxs_ap=idx16[:, o16:o16 + w16],
                    num_idxs=n,
                    num_idxs_reg=n,
                    elem_size=_PROW,
                    queue_num=q,
                ).then_inc(s_gq[c], 16)

    nc.finalize()
    _nc_cache = nc
    return nc


def kernel(heatmap, image):
    from concourse.bass_utils import run_bass_kernel_spmd

    heatmap = np.ascontiguousarray(np.asarray(heatmap), dtype=np.float32)
    image = np.ascontiguousarray(np.asarray(image), dtype=np.float32)
    assert heatmap.shape == (_B, 448, 448, 1)
    assert image.shape == (_B, 448, 448, 3)

    nc = build_program()
    in_maps = [
        {
            "heatmap": heatmap[c * _B_LOC:(c + 1) * _B_LOC],
            "image": image[c * _B_LOC:(c + 1) * _B_LOC],
        }
        for c in range(_N_CORES)
    ]
    res = run_bass_kernel_spmd(nc, in_maps, list(range(_N_CORES)))
    outs = [res.results[c]["out"] for c in range(_N_CORES)]
    return np.concatenate(outs, axis=0)
